# revision 26
# baseline (speedup 1.0000x reference)
"""Segment-sum (jax.ops.segment_sum(H, X_node, num_segments=V)) on 8 trn2
NeuronCores.

Strategy
--------
Host-side sharding: rows are routed to the core that owns their output
segment slice (V split into 8 slices of 98 blocks x 128 segments), and
bucketed by 128-segment block.  This is the all-to-all of the classic
"local partial + all-reduce" decomposition folded into input distribution:
every core ends up with a disjoint output slice, so no reduce is needed.

Device-side compute: for each 128-token tile (all tokens of one block), a
one-hot matrix onehot[token, seg_lo7] is built on the vector engine
(is_equal against an iota row) and the tensor engine accumulates
  psum[seg, :] += onehot.T @ [H_hi | H_lo]
over the block's tiles.  H is shipped as a bf16 hi/lo split (same bytes as
f32) so the matmul runs at bf16 rate while keeping ~1e-5 relative accuracy
(products are exact, accumulation is fp32 in PSUM).  Each block's psum is
flushed as hi+lo into an SBUF staging tile, DMA'd out once at the end.

All blocks are padded to a common tile count (Tmax) so the 8 cores run one
identical static program.
"""

import math
import sys

sys.path.insert(0, "/opt/trn_rl_repo")

import numpy as np
import ml_dtypes

import concourse.bacc as bacc
import concourse.mybir as mybir
import concourse.tile as tile
from concourse.bass_utils import run_bass_kernel_spmd

P = 128          # partitions / tokens per tile / segments per block
D = 64           # feature dim
V = 100000       # number of segments
NCORES = 8
NB = 98          # blocks per core (8*98*128 = 100352 >= V)
SLICE = NB * P   # segments per core
TC = 32          # tiles per DMA chunk

_BUILD_CACHE: dict = {}


OH_BATCH = 4   # legacy (batched-TT path removed); keep tc % OH_BATCH == 0
OP_BUFS = 24   # onehot tiles in flight (DVE -> PE pipeline depth)
HP_BUFS = 4    # H chunk buffers
FLUSH_ENGINE = "vector"  # vector | scalar (PSUM hi-half copy engine)


def _build(nb: int, tmax: int, tc: int, nchunks: int, variant: str = "full"):
    """Static SPMD program: nb blocks, tmax tiles per block, chunks of tc
    tiles.  variant: full | dmaonly | nodve | nope (ablation timing only)
    | tsN (timing: N-times-unrolled compute, tiny output)."""
    key = (nb, tmax, tc, nchunks, variant)
    if key in _BUILD_CACHE:
        return _BUILD_CACHE[key]
    reps = 1
    small_out = False
    if variant.startswith("ts"):
        reps = int(variant[2:])
        small_out = True
        variant = "full"
    ntiles = nb * tmax
    assert nchunks * tc >= ntiles
    assert tc % OH_BATCH == 0
    nc = bacc.Bacc("TRN2")
    hin = nc.dram_tensor("h", [nchunks, P, tc * 2 * D], mybir.dt.bfloat16,
                         kind="ExternalInput")
    lin = nc.dram_tensor("lo", [P, nchunks * tc], mybir.dt.float32,
                         kind="ExternalInput")
    iin = nc.dram_tensor("iota", [P, P], mybir.dt.bfloat16,
                         kind="ExternalInput")
    out = nc.dram_tensor("out", [P, D if small_out else nb * D],
                         mybir.dt.float32, kind="ExternalOutput")

    with tile.TileContext(nc) as tc_ctx:
        with (
            tc_ctx.tile_pool(name="hp", bufs=HP_BUFS) as hp,
            tc_ctx.tile_pool(name="op", bufs=OP_BUFS) as op,
            tc_ctx.tile_pool(name="pp", bufs=8, space="PSUM") as pp,
            tc_ctx.tile_pool(name="cp", bufs=1) as cp,
        ):
            iota = cp.tile([P, P], mybir.dt.bfloat16, tag="iota")
            nc.sync.dma_start(iota[:], iin[:])
            const_oh = cp.tile([P, P], mybir.dt.bfloat16, tag="constoh")
            nc.vector.tensor_tensor(
                out=const_oh[:], in0=iota[:], in1=iota[:],
                op=mybir.AluOpType.is_equal)
            ostage = cp.tile([P, nb * D], mybir.dt.float32, tag="ostage")
            if variant in ("dmaonly", "nope"):
                nc.gpsimd.memset(ostage[:], 0.0)
            lall = cp.tile([P, nchunks * tc], mybir.dt.float32, tag="lall")
            nc.sync.dma_start(lall[:], lin[:])
            psum = None
            for _rep, ch in ((r, c) for r in range(reps)
                             for c in range(nchunks)):
                htile = hp.tile([P, tc * 2 * D], mybir.dt.bfloat16)
                nc.sync.dma_start(htile[:], hin[ch])
                if variant == "dmaonly":
                    continue
                for k in range(tc):
                    g = ch * tc + k
                    if g >= ntiles:
                        break
                    b, j = divmod(g, tmax)
                    ohtile = None
                    if variant != "nodve":
                        # tensor_scalar: iota row packed bf16 (in0), lo7 as
                        # per-partition scalar -> DVE 4x_2p perf mode
                        ohtile = op.tile([P, P], mybir.dt.bfloat16)
                        nc.vector.tensor_scalar(
                            out=ohtile[:],
                            in0=iota[:],
                            scalar1=lall[:, g:g + 1],
                            scalar2=None,
                            op0=mybir.AluOpType.is_equal,
                        )
                    if variant == "nope":
                        continue
                    oh = (const_oh[:] if variant == "nodve" else ohtile[:])
                    if j == 0:
                        psum = pp.tile([P, 2 * D], mybir.dt.float32)
                    nc.tensor.matmul(
                        psum[:],
                        lhsT=oh,
                        rhs=htile[:, k * 2 * D:(k + 1) * 2 * D],
                        start=(j == 0),
                        stop=(j == tmax - 1),
                    )
                    if j == tmax - 1:
                        # DVE may read only one PSUM operand per instruction
                        if FLUSH_ENGINE == "scalar":
                            nc.scalar.copy(
                                out=ostage[:, b * D:(b + 1) * D],
                                in_=psum[:, :D],
                            )
                        else:
                            nc.vector.tensor_copy(
                                out=ostage[:, b * D:(b + 1) * D],
                                in_=psum[:, :D],
                            )
                        nc.vector.tensor_add(
                            out=ostage[:, b * D:(b + 1) * D],
                            in0=ostage[:, b * D:(b + 1) * D],
                            in1=psum[:, D:2 * D],
                        )
                        # stream completed quarters of ostage out early so
                        # the final store overlaps compute
                        if not small_out and _rep == reps - 1:
                            q = nb // 4
                            if b + 1 in (q, 2 * q, 3 * q):
                                s = (b + 1 - q) * D
                                nc.sync.dma_start(
                                    out[:, s:(b + 1) * D],
                                    ostage[:, s:(b + 1) * D])
            if small_out:
                nc.sync.dma_start(out[:], ostage[:, :D])
            else:
                q = nb // 4
                nc.sync.dma_start(out[:, 3 * q * D:], ostage[:, 3 * q * D:])
    nc.finalize()
    _BUILD_CACHE[key] = nc
    return nc


def _pack_blocks(seg: np.ndarray, v: int, nblocks: int):
    """Assign segments to (block, slot) balancing token counts per block
    (LPT greedy with 128-slot capacity).  Returns blk_of_seg, slot_of_seg."""
    import heapq

    cnt = np.bincount(seg, minlength=v)
    order = np.argsort(-cnt, kind="stable")
    heap = [(0, b) for b in range(nblocks)]
    space = np.full(nblocks, P, np.int32)
    blk_of = np.empty(v, np.int32)
    slot_of = np.empty(v, np.int32)
    for s in order:
        while True:
            load, b = heapq.heappop(heap)
            if space[b] > 0:
                break
        blk_of[s] = b
        slot_of[s] = P - space[b]
        space[b] -= 1
        heapq.heappush(heap, (load + int(cnt[s]), b))
    return blk_of, slot_of


def _host_prep(H: np.ndarray, seg: np.ndarray, ncores: int, nb: int,
               tc: int):
    """Route rows to (core, block) buckets and build device input arrays."""
    n, d = H.shape
    v = ncores * nb * P
    blk_of, slot_of = _pack_blocks(seg, v, ncores * nb)
    blk = blk_of[seg]                          # balanced block id per token
    order = np.argsort(blk, kind="stable")
    cnt = np.bincount(blk, minlength=ncores * nb)
    assert cnt.shape[0] == ncores * nb
    tmax = max(1, int(math.ceil(cnt.max() / P)))
    ntiles = nb * tmax
    nchunks = int(math.ceil(ntiles / tc))
    rows_pad = nchunks * tc * P
    starts = np.zeros(ncores * nb + 1, np.int64)
    np.cumsum(cnt, out=starts[1:])

    iota = np.broadcast_to(np.arange(P), (P, P)).astype(ml_dtypes.bfloat16)
    in_maps = []
    for c in range(ncores):
        rows = np.zeros((rows_pad, d), np.float32)
        lo = np.zeros(rows_pad, np.float32)
        for lb in range(nb):
            b = c * nb + lb
            k0, k1 = starts[b], starts[b + 1]
            if k1 == k0:
                continue
            dst0 = lb * tmax * P
            idx = order[k0:k1]
            rows[dst0:dst0 + (k1 - k0)] = H[idx]
            lo[dst0:dst0 + (k1 - k0)] = slot_of[seg[idx]].astype(np.float32)
        hi = rows.astype(ml_dtypes.bfloat16)
        lo_res = (rows - hi.astype(np.float32)).astype(ml_dtypes.bfloat16)
        hl = np.concatenate(
            [hi.reshape(rows_pad // P, P, d), lo_res.reshape(rows_pad // P, P, d)],
            axis=-1,
        )  # [ntiles_pad, P, 2D]
        hdev = np.ascontiguousarray(
            hl.reshape(nchunks, tc, P, 2 * d).transpose(0, 2, 1, 3)
        ).reshape(nchunks, P, tc * 2 * d)
        lodev = np.ascontiguousarray(lo.reshape(nchunks * tc, P).T)
        in_maps.append({"h": hdev, "lo": lodev, "iota": iota})
    outperm = blk_of.astype(np.int64) * P + slot_of  # seg -> output slot
    return in_maps, tmax, nchunks, outperm


def _unshard(results, ncores: int, nb: int, outperm: np.ndarray) -> np.ndarray:
    full = np.empty((ncores * nb * P, D), np.float32)
    for c in range(ncores):
        o = np.asarray(results[c]["out"]).reshape(P, nb, D)
        full[c * nb * P:(c + 1) * nb * P] = (
            o.transpose(1, 0, 2).reshape(nb * P, D)
        )
    return full[outperm]


def _run(H, X_node, trace=False, trace_kwargs=None):
    H = np.ascontiguousarray(np.asarray(H, dtype=np.float32))
    seg = np.asarray(X_node).astype(np.int64)
    in_maps, tmax, nchunks, outperm = _host_prep(H, seg, NCORES, NB, TC)
    nc = _build(NB, tmax, TC, nchunks)
    kwargs = {}
    if trace:
        kwargs = dict(trace=True, trace_cores=list(range(NCORES)),
                      stitch_traces=False)
        if trace_kwargs:
            kwargs.update(trace_kwargs)
    res = run_bass_kernel_spmd(nc, in_maps, core_ids=list(range(NCORES)),
                               **kwargs)
    out = _unshard(res.results, NCORES, NB, outperm[:V])
    return out, res


def kernel(H, X_node) -> np.ndarray:
    out, _ = _run(H, X_node, trace=False)
    return out


if __name__ == "__main__":
    # tiny smoke test on hardware (all 8 cores, small V')
    rng = np.random.default_rng(0)
    n_small, v_small, nb_small, tc_small = 6000, NCORES * 2 * P, 2, 4
    Hs = rng.standard_normal((n_small, D)).astype(np.float32)
    segs = rng.integers(0, v_small, size=n_small).astype(np.int64)
    in_maps, tmax, nchunks, outperm = _host_prep(Hs, segs, NCORES, nb_small,
                                                 tc_small)
    nc = _build(nb_small, tmax, tc_small, nchunks)
    res = run_bass_kernel_spmd(nc, in_maps, core_ids=list(range(NCORES)))
    got = _unshard(res.results, NCORES, nb_small, outperm[:v_small])
    exp = np.zeros((v_small, D), np.float32)
    np.add.at(exp, segs, Hs)
    err = np.abs(got - exp).max() / max(1e-9, np.abs(exp).max())
    print(f"smoke: tmax={tmax} nchunks={nchunks} max-rel-err={err:.3e}")
    assert err < 1e-4, "smoke test failed"
    print("SMOKE PASS")


# revision 27
# speedup vs baseline: 9.2278x; 9.2278x over previous
"""Segment-sum (jax.ops.segment_sum(H, X_node, num_segments=V)) on 8 trn2
NeuronCores.

Strategy
--------
Host-side sharding: segments are bin-packed (LPT, 128 slots per block)
into 8*98 blocks with balanced token counts, and each row is routed to
the core owning its segment's block.  This folds the all-to-all of the
classic "local partial + all-reduce" decomposition into input
distribution: every core produces a disjoint set of output rows, so no
device reduce is needed (collectives measure ~32-60 GB/s here); the host
applies the inverse segment permutation on the gathered outputs.

Device-side compute: for each 128-token tile (all tokens of one block), a
one-hot matrix onehot[token, slot] is built on the vector engine
(tensor_scalar is_equal against a packed bf16 iota row -> DVE 4x mode)
and the tensor engine accumulates
  psum[slot, :] += onehot.T @ [H_hi | H_lo]
over the block's tiles.  H is shipped as a bf16 hi/lo split (same bytes as
f32) so the matmul runs at bf16 rate while keeping ~2.5e-6 l2 relative
error (products are exact, accumulation is fp32 in PSUM).  Each block's
psum is flushed as hi+lo into an SBUF staging tile which streams out in
quarters overlapped with compute.

Bin-packing makes every block ~equal, so the common per-block tile count
(Tmax, typically 20) carries ~0.4% padding, and the 8 cores run one
identical static program.  Cost model (validated on HW): ~240us/core vs a
~197us DMA floor for the 63MB/core streamed.
"""

import math
import sys

sys.path.insert(0, "/opt/trn_rl_repo")

import numpy as np
import ml_dtypes

import concourse.bacc as bacc
import concourse.mybir as mybir
import concourse.tile as tile
from concourse.bass_utils import run_bass_kernel_spmd

P = 128          # partitions / tokens per tile / segments per block
D = 64           # feature dim
V = 100000       # number of segments
NCORES = 8
NB = 98          # blocks per core (8*98*128 = 100352 >= V)
SLICE = NB * P   # segments per core
TC = 32          # tiles per DMA chunk

_BUILD_CACHE: dict = {}


OH_BATCH = 4   # legacy (batched-TT path removed); keep tc % OH_BATCH == 0
OP_BUFS = 24   # onehot tiles in flight (DVE -> PE pipeline depth)
HP_BUFS = 4    # H chunk buffers
FLUSH_ENGINE = "vector"  # vector | scalar (PSUM hi-half copy engine)


def _build(nb: int, tmax: int, tc: int, nchunks: int, variant: str = "full"):
    """Static SPMD program: nb blocks, tmax tiles per block, chunks of tc
    tiles.  variant: full | dmaonly | nodve | nope (ablation timing only)
    | tsN (timing: N-times-unrolled compute, tiny output)."""
    key = (nb, tmax, tc, nchunks, variant)
    if key in _BUILD_CACHE:
        return _BUILD_CACHE[key]
    reps = 1
    small_out = False
    if variant.startswith("ts"):
        reps = int(variant[2:])
        small_out = True
        variant = "full"
    ntiles = nb * tmax
    assert nchunks * tc >= ntiles
    assert tc % OH_BATCH == 0
    nc = bacc.Bacc("TRN2")
    hin = nc.dram_tensor("h", [nchunks, P, tc * 2 * D], mybir.dt.bfloat16,
                         kind="ExternalInput")
    lin = nc.dram_tensor("lo", [P, nchunks * tc], mybir.dt.float32,
                         kind="ExternalInput")
    iin = nc.dram_tensor("iota", [P, P], mybir.dt.bfloat16,
                         kind="ExternalInput")
    out = nc.dram_tensor("out", [P, D if small_out else nb * D],
                         mybir.dt.float32, kind="ExternalOutput")

    with tile.TileContext(nc) as tc_ctx:
        with (
            tc_ctx.tile_pool(name="hp", bufs=HP_BUFS) as hp,
            tc_ctx.tile_pool(name="op", bufs=OP_BUFS) as op,
            tc_ctx.tile_pool(name="pp", bufs=8, space="PSUM") as pp,
            tc_ctx.tile_pool(name="cp", bufs=1) as cp,
        ):
            iota = cp.tile([P, P], mybir.dt.bfloat16, tag="iota")
            nc.sync.dma_start(iota[:], iin[:])
            const_oh = cp.tile([P, P], mybir.dt.bfloat16, tag="constoh")
            nc.vector.tensor_tensor(
                out=const_oh[:], in0=iota[:], in1=iota[:],
                op=mybir.AluOpType.is_equal)
            ostage = cp.tile([P, nb * D], mybir.dt.float32, tag="ostage")
            if variant in ("dmaonly", "nope"):
                nc.gpsimd.memset(ostage[:], 0.0)
            lall = cp.tile([P, nchunks * tc], mybir.dt.float32, tag="lall")
            nc.sync.dma_start(lall[:], lin[:])
            psum = None
            for _rep, ch in ((r, c) for r in range(reps)
                             for c in range(nchunks)):
                htile = hp.tile([P, tc * 2 * D], mybir.dt.bfloat16)
                nc.sync.dma_start(htile[:], hin[ch])
                if variant == "dmaonly":
                    continue
                for k in range(tc):
                    g = ch * tc + k
                    if g >= ntiles:
                        break
                    b, j = divmod(g, tmax)
                    ohtile = None
                    if variant != "nodve":
                        # tensor_scalar: iota row packed bf16 (in0), lo7 as
                        # per-partition scalar -> DVE 4x_2p perf mode
                        ohtile = op.tile([P, P], mybir.dt.bfloat16)
                        nc.vector.tensor_scalar(
                            out=ohtile[:],
                            in0=iota[:],
                            scalar1=lall[:, g:g + 1],
                            scalar2=None,
                            op0=mybir.AluOpType.is_equal,
                        )
                    if variant == "nope":
                        continue
                    oh = (const_oh[:] if variant == "nodve" else ohtile[:])
                    if j == 0:
                        psum = pp.tile([P, 2 * D], mybir.dt.float32)
                    nc.tensor.matmul(
                        psum[:],
                        lhsT=oh,
                        rhs=htile[:, k * 2 * D:(k + 1) * 2 * D],
                        start=(j == 0),
                        stop=(j == tmax - 1),
                    )
                    if j == tmax - 1:
                        # DVE may read only one PSUM operand per instruction
                        if FLUSH_ENGINE == "scalar":
                            nc.scalar.copy(
                                out=ostage[:, b * D:(b + 1) * D],
                                in_=psum[:, :D],
                            )
                        else:
                            nc.vector.tensor_copy(
                                out=ostage[:, b * D:(b + 1) * D],
                                in_=psum[:, :D],
                            )
                        nc.vector.tensor_add(
                            out=ostage[:, b * D:(b + 1) * D],
                            in0=ostage[:, b * D:(b + 1) * D],
                            in1=psum[:, D:2 * D],
                        )
                        # stream completed quarters of ostage out early so
                        # the final store overlaps compute
                        if not small_out and _rep == reps - 1:
                            q = nb // 4
                            if b + 1 in (q, 2 * q, 3 * q):
                                s = (b + 1 - q) * D
                                nc.sync.dma_start(
                                    out[:, s:(b + 1) * D],
                                    ostage[:, s:(b + 1) * D])
            if small_out:
                nc.sync.dma_start(out[:], ostage[:, :D])
            else:
                q = nb // 4
                nc.sync.dma_start(out[:, 3 * q * D:], ostage[:, 3 * q * D:])
    nc.finalize()
    _BUILD_CACHE[key] = nc
    return nc


def _pack_blocks(seg: np.ndarray, v: int, nblocks: int):
    """Assign segments to (block, slot) balancing token counts per block
    (LPT greedy with 128-slot capacity).  Returns blk_of_seg, slot_of_seg."""
    import heapq

    cnt = np.bincount(seg, minlength=v)
    order = np.argsort(-cnt, kind="stable")
    heap = [(0, b) for b in range(nblocks)]
    space = np.full(nblocks, P, np.int32)
    blk_of = np.empty(v, np.int32)
    slot_of = np.empty(v, np.int32)
    for s in order:
        while True:
            load, b = heapq.heappop(heap)
            if space[b] > 0:
                break
        blk_of[s] = b
        slot_of[s] = P - space[b]
        space[b] -= 1
        heapq.heappush(heap, (load + int(cnt[s]), b))
    return blk_of, slot_of


def _host_prep(H: np.ndarray, seg: np.ndarray, ncores: int, nb: int,
               tc: int):
    """Route rows to (core, block) buckets and build device input arrays."""
    n, d = H.shape
    v = ncores * nb * P
    blk_of, slot_of = _pack_blocks(seg, v, ncores * nb)
    blk = blk_of[seg]                          # balanced block id per token
    order = np.argsort(blk, kind="stable")
    cnt = np.bincount(blk, minlength=ncores * nb)
    assert cnt.shape[0] == ncores * nb
    tmax = max(1, int(math.ceil(cnt.max() / P)))
    ntiles = nb * tmax
    nchunks = int(math.ceil(ntiles / tc))
    rows_pad = nchunks * tc * P
    starts = np.zeros(ncores * nb + 1, np.int64)
    np.cumsum(cnt, out=starts[1:])

    iota = np.broadcast_to(np.arange(P), (P, P)).astype(ml_dtypes.bfloat16)
    in_maps = []
    for c in range(ncores):
        rows = np.zeros((rows_pad, d), np.float32)
        lo = np.zeros(rows_pad, np.float32)
        for lb in range(nb):
            b = c * nb + lb
            k0, k1 = starts[b], starts[b + 1]
            if k1 == k0:
                continue
            dst0 = lb * tmax * P
            idx = order[k0:k1]
            rows[dst0:dst0 + (k1 - k0)] = H[idx]
            lo[dst0:dst0 + (k1 - k0)] = slot_of[seg[idx]].astype(np.float32)
        hi = rows.astype(ml_dtypes.bfloat16)
        lo_res = (rows - hi.astype(np.float32)).astype(ml_dtypes.bfloat16)
        hl = np.concatenate(
            [hi.reshape(rows_pad // P, P, d), lo_res.reshape(rows_pad // P, P, d)],
            axis=-1,
        )  # [ntiles_pad, P, 2D]
        hdev = np.ascontiguousarray(
            hl.reshape(nchunks, tc, P, 2 * d).transpose(0, 2, 1, 3)
        ).reshape(nchunks, P, tc * 2 * d)
        lodev = np.ascontiguousarray(lo.reshape(nchunks * tc, P).T)
        in_maps.append({"h": hdev, "lo": lodev, "iota": iota})
    outperm = blk_of.astype(np.int64) * P + slot_of  # seg -> output slot
    return in_maps, tmax, nchunks, outperm


def _unshard(results, ncores: int, nb: int, outperm: np.ndarray) -> np.ndarray:
    full = np.empty((ncores * nb * P, D), np.float32)
    for c in range(ncores):
        o = np.asarray(results[c]["out"]).reshape(P, nb, D)
        full[c * nb * P:(c + 1) * nb * P] = (
            o.transpose(1, 0, 2).reshape(nb * P, D)
        )
    return full[outperm]


def _run(H, X_node, trace=False, trace_kwargs=None):
    H = np.ascontiguousarray(np.asarray(H, dtype=np.float32))
    seg = np.asarray(X_node).astype(np.int64)
    in_maps, tmax, nchunks, outperm = _host_prep(H, seg, NCORES, NB, TC)
    nc = _build(NB, tmax, TC, nchunks)
    kwargs = {}
    if trace:
        kwargs = dict(trace=True, trace_cores=list(range(NCORES)),
                      stitch_traces=False)
        if trace_kwargs:
            kwargs.update(trace_kwargs)
    res = run_bass_kernel_spmd(nc, in_maps, core_ids=list(range(NCORES)),
                               **kwargs)
    out = _unshard(res.results, NCORES, NB, outperm[:V])
    return out, res


def kernel(H, X_node) -> np.ndarray:
    out, _ = _run(H, X_node, trace=False)
    return out


if __name__ == "__main__":
    # tiny smoke test on hardware (all 8 cores, small V')
    rng = np.random.default_rng(0)
    n_small, v_small, nb_small, tc_small = 6000, NCORES * 2 * P, 2, 4
    Hs = rng.standard_normal((n_small, D)).astype(np.float32)
    segs = rng.integers(0, v_small, size=n_small).astype(np.int64)
    in_maps, tmax, nchunks, outperm = _host_prep(Hs, segs, NCORES, nb_small,
                                                 tc_small)
    nc = _build(nb_small, tmax, tc_small, nchunks)
    res = run_bass_kernel_spmd(nc, in_maps, core_ids=list(range(NCORES)))
    got = _unshard(res.results, NCORES, nb_small, outperm[:v_small])
    exp = np.zeros((v_small, D), np.float32)
    np.add.at(exp, segs, Hs)
    err = np.abs(got - exp).max() / max(1e-9, np.abs(exp).max())
    print(f"smoke: tmax={tmax} nchunks={nchunks} max-rel-err={err:.3e}")
    assert err < 1e-4, "smoke test failed"
    print("SMOKE PASS")
